# revision 69
# baseline (speedup 1.0000x reference)
"""Trainium2 Bass kernel for nn_Attn_30820685316537 (segment_reduce attention).

Reference computation (per batch b):
    score = output @ context^T                     [Q, S]
    avg   = per-segment mean of score over S, broadcast back
    align = softmax(avg, axis=S)                   [Q, S]
    ac    = align @ context                        [Q, D]
    out   = tanh(concat(ac, output) @ W^T + bias)  [Q, D]
    returns (out, align)

Everything factors through rank-64 segment space (avg is constant within each
contiguous segment).  With Cavg[n, d] = (1/cnt_n) * sum_{s in seg n} C[s, d]:
    segavg[q, n] = O[q, :] . Cavg[n, :]
    u[q, n]      = exp(segavg + ln cnt_n - max)        (cnt-weighted softmax)
    urn[q, n]    = u / sum_n u                         (per-segment align mass)
    align[q, s]  = urn[q, seg(s)] / cnt_{seg(s)}       (host-side gather)
    ac[q, :]     = urn @ Cavg
    out          = tanh(urn @ (Cavg @ W1) + O @ W2 + bias)
where W^T = [W1; W2].  P = Cavg @ W1 is a [64, D] matrix, so the output GEMM
is O @ W2 (K=1024) plus a K=65 rank-64 correction (bias folded in as P's 65th
row) instead of the K=2048 concat GEMM.

Device schedule per batch:
  front: Cavg  = ohi^T @ ctx (one-hot matmul, invc pre-folded on host),
         CavgT = 8 PE transposes, segavgT = CavgT^T @ O^T as 8 wide (N=512)
         matmuls, 4 transposes back to [q, n], then all four q-tiles'
         softmaxes run up front on DVE/ACT (decoupled from the GEMM stream),
         P = CavgT^T @ W1 in fp8 (x4096 scaling keeps fp8 normals).
  qloop (per 128-row q-tile): O@W2 lo columns, then hi columns (the W2
         column-halves stream from HBM in that order); urn is transposed
         (batched, one PSUM tile) and applied (K=65) into the same PSUM
         accumulators; tanh + DMA out per half.
Two batches per core are software-pipelined; GEMMs fp16 (fp32 PSUM
accumulation), P-path fp8.  align is reconstructed on the host by a pure
gather of the device-computed urn masses.  DMA rings are byte-balanced and
ordered by need-time (ctx0/W early, batch-1 inputs behind batch-0 on the
same queues); warm-up/keep-alive matmuls hold the PE's HAM clock-gate at
full rate through the input-paced ramp.

Sharding: data-parallel over batch B=16 across 8 NeuronCores; W replicated.
"""
import numpy as np
from contextlib import ExitStack

B, Q, S, D = 16, 512, 1024, 1024
NSEG = 64
NCORES = 8
BPC = B // NCORES          # batches per core
QT = Q // 128              # 4 q-tiles
ST = S // 128              # 8 s-chunks
DT = D // 128              # 8 d-chunks

_CACHE = {}


def _build_nc():
    import concourse.bacc as bacc
    import concourse.tile as tile
    import concourse.mybir as mybir

    f32 = mybir.dt.float32
    f16 = mybir.dt.float16
    f8 = mybir.dt.float8e4

    nc = bacc.Bacc("TRN2", target_bir_lowering=False, debug=False,
                   enable_asserts=False, num_devices=NCORES)

    ctx_in = nc.dram_tensor("ctx_in", [BPC, S, D], f16, kind="ExternalInput")
    ott_in = nc.dram_tensor("ott_in", [BPC, D, Q], f16, kind="ExternalInput")   # O^T
    w1_in = nc.dram_tensor("w1_in", [D, D], f8, kind="ExternalInput")           # WT[:D]
    w2_in = nc.dram_tensor("w2_in", [D, D], f16, kind="ExternalInput")          # WT[D:]
    ohi_in = nc.dram_tensor("ohi_in", [BPC, 128, ST * NSEG], f16, kind="ExternalInput")
    lnc_in = nc.dram_tensor("lnc_in", [BPC, 128, NSEG], f32, kind="ExternalInput")
    bias_in = nc.dram_tensor("bias_in", [1, D], f16, kind="ExternalInput")
    identh_in = nc.dram_tensor("identh_in", [128, 128], f16, kind="ExternalInput")

    out_o = nc.dram_tensor("out_o", [BPC, Q, D], f16, kind="ExternalOutput")
    urn_o = nc.dram_tensor("urn_o", [BPC, Q, NSEG], f16, kind="ExternalOutput")

    Exp = mybir.ActivationFunctionType.Exp
    Tanh = mybir.ActivationFunctionType.Tanh

    with tile.TileContext(nc) as tc, ExitStack() as ectx:
        consts = ectx.enter_context(tc.tile_pool(name="consts", bufs=1))
        inp = ectx.enter_context(tc.tile_pool(name="inp", bufs=2))
        front = ectx.enter_context(tc.tile_pool(name="front", bufs=2))
        sm = ectx.enter_context(tc.tile_pool(name="sm", bufs=3))
        stage = ectx.enter_context(tc.tile_pool(name="stage", bufs=2))

        # PSUM: exactly 8 banks (2 + 1 + 2 + 2 + 1 junk).
        ps64 = ectx.enter_context(tc.tile_pool(name="ps64", bufs=2, space="PSUM"))
        ps_t = ectx.enter_context(tc.tile_pool(name="ps_t", bufs=1, space="PSUM"))
        ps_lo = ectx.enter_context(tc.tile_pool(name="ps_lo", bufs=2, space="PSUM"))
        ps_hi = ectx.enter_context(tc.tile_pool(name="ps_hi", bufs=2, space="PSUM"))
        ps_j = ectx.enter_context(tc.tile_pool(name="ps_j", bufs=1, space="PSUM"))

        # ---- const loads (gpsimd ring: identh first, then W1, then W2-lo) ----
        identh = consts.tile([128, 128], f16, tag="identh")
        nc.gpsimd.dma_start(identh[:], identh_in.ap())
        bias_sb = consts.tile([1, D], f16, tag="bias")
        nc.gpsimd.dma_start(bias_sb[:], bias_in.ap())

        w1_all = consts.tile([128, DT * D], f8, tag="w1")      # [p, (d f)] fp8
        w1v = w1_all[:].rearrange("p (c f) -> p c f", f=D)
        w1s = w1_in.ap().rearrange("(c p) f -> p c f", p=128)
        for t in range(2):
            nc.gpsimd.dma_start(w1v[:, 4 * t:4 * (t + 1), :], w1s[:, 4 * t:4 * (t + 1), :])

        w2_all = consts.tile([128, DT * D], f16, tag="w2")
        w2v = w2_all[:].rearrange("p (c f) -> p c f", f=D)
        w2s = w2_in.ap().rearrange("(c p) f -> p c f", p=128)
        # lo columns of every chunk first (feeds the qlo pass ~6µs before
        # the qhi pass needs the hi columns; w2-hi rides the sync ring).
        nc.gpsimd.dma_start(w2v[:, 0:4, 0:512], w2s[:, 0:4, 0:512])
        nc.gpsimd.dma_start(w2v[:, 4:8, 0:512], w2s[:, 4:8, 0:512])

        Xax = mybir.AxisListType.X

        state = [dict() for _ in range(BPC)]

        def emit_loads(b, eng):
            st = state[b]
            ohi = inp.tile([128, ST * NSEG], f16, tag="ohi")
            eng.dma_start(ohi[:], ohi_in.ap()[b])
            lnc = inp.tile([128, NSEG], f32, tag="lnc")
            eng.dma_start(lnc[:], lnc_in.ap()[b])
            ctx_all = inp.tile([128, ST * D], f16, tag="ctx")   # [p, (i d)]
            cv = ctx_all[:].rearrange("p (c d) -> p c d", d=D)
            cs = ctx_in.ap()[b].rearrange("(c p) d -> p c d", p=128)
            st["ohi"], st["lnc"], st["ctx"] = ohi, lnc, ctx_all
            return cv, cs

        def emit_load_ott(b, eng):
            st = state[b]
            ott_all = inp.tile([128, DT * Q], f16, tag="ott")   # [p, (d q)]
            ov = ott_all[:].rearrange("p (c q) -> p c q", q=Q)
            os_ = ott_in.ap()[b].rearrange("(c p) q -> p c q", p=128)
            eng.dma_start(ov[:, 0:4, :], os_[:, 0:4, :])
            eng.dma_start(ov[:, 4:8, :], os_[:, 4:8, :])
            st["ott"] = ott_all

        junk = [None]

        def emit_warmup():
            # Dense PE work during the initial DMA wait so HAM un-throttles
            # before the real stream starts (identh arrives in the first µs).
            junk[0] = ps_j.tile([128, 512], f32, tag="junk", name="junkpw")
            pw = junk[0]
            for r in range(32):
                nc.tensor.matmul(pw[:, 0:128], identh[:], identh[:],
                                 start=(r == 0), stop=(r == 31))

        def emit_rewarm(n):
            # Keep-alive matmuls emitted just before a known input-wait point:
            # they execute during the DMA wait and flip HAM back to full clock
            # before the dense GEMM stream starts.
            pw = junk[0]
            for r in range(n):
                nc.tensor.matmul(pw[:, 0:128], identh[:], identh[:],
                                 start=True, stop=True)

        def emit_csum_mm(b, i):
            st = state[b]
            ohi, ctx_all = st["ohi"], st["ctx"]
            if i == 0:
                st["cs_lo"] = ps64.tile([64, 512], f32, tag="a64", name="cs_lo")
                st["cs_hi"] = ps64.tile([64, 512], f32, tag="a64", name="cs_hi")
            oh_i = ohi[:, NSEG * i:NSEG * (i + 1)]
            nc.tensor.matmul(st["cs_lo"][:], oh_i, ctx_all[:, D * i:D * i + 512],
                             start=(i == 0), stop=(i == ST - 1))
            nc.tensor.matmul(st["cs_hi"][:], oh_i, ctx_all[:, D * i + 512:D * (i + 1)],
                             start=(i == 0), stop=(i == ST - 1))

        def emit_csum(b, lo=0, hi=ST):
            st = state[b]
            for i in range(lo, hi):
                if b == 0:
                    emit_rewarm(6)   # fill the ctx-arrival pacing gap
                emit_csum_mm(b, i)
            if hi < ST:
                return
            cs_lo, cs_hi = st["cs_lo"], st["cs_hi"]
            csum = front.tile([64, D], f16, tag="csum")
            nc.vector.tensor_copy(csum[:, 0:512], cs_lo[:])
            nc.vector.tensor_copy(csum[:, 512:1024], cs_hi[:])

            # CavgT packed [128, (d n)] via 8 PE transposes
            csumt = front.tile([128, DT * NSEG], f16, tag="csumt")
            pt = ps_t.tile([128, 1024], f16, tag="tp")
            pt2 = ps_t.tile([128, 1024], f16, tag="tp")
            for d in range(DT):
                po = pt if d < 4 else pt2
                nc.tensor.transpose(po[:, 64 * (d % 4):64 * (d % 4 + 1)],
                                    csum[0:64, 128 * d:128 * (d + 1)],
                                    identh[0:64, 0:64])
            nc.vector.tensor_copy(csumt[:, 0:256], pt[:, 0:256])
            nc.vector.tensor_copy(csumt[:, 256:512], pt2[:, 0:256])
            st["csumt"] = csumt
            csumt8 = front.tile([128, DT * NSEG], f8, tag="csumt8")
            nc.vector.tensor_scalar_mul(csumt8[:, 0:256], pt[:, 0:256], 16.0)
            nc.vector.tensor_scalar_mul(csumt8[:, 256:512], pt2[:, 0:256], 16.0)
            st["csumt8"] = csumt8

            urt = front.tile([65, Q], f16, tag="urt")
            nc.vector.memset(urt[64:65, :], 1.0)
            st["urt"] = urt

        def emit_segavg(b):
            # segavgT[n, q] = Cavg @ O^T as 8 wide matmuls, then transpose to
            # [q, n] and run all four q-tiles' softmaxes up front (decoupled
            # from the O@W2 stream).
            st = state[b]
            csumt, ott, lnc = st["csumt"], st["ott"], st["lnc"]
            sgt = ps64.tile([64, Q], f32, tag="a64")
            for d in range(DT):
                nc.tensor.matmul(sgt[:], csumt[:, NSEG * d:NSEG * (d + 1)],
                                 ott[:, Q * d:Q * (d + 1)],
                                 start=(d == 0), stop=(d == DT - 1))
            sgt_sb = front.tile([64, Q], f16, tag="sgt")
            nc.vector.tensor_copy(sgt_sb[:], sgt[:])
            sgtT = ps_t.tile([128, 1024], f16, tag="tp")
            for j in range(QT):
                nc.tensor.transpose(sgtT[:, 64 * j:64 * (j + 1)],
                                    sgt_sb[0:64, 128 * j:128 * (j + 1)],
                                    identh[0:64, 0:64])
            for j in range(QT):
                sg2 = sm.tile([128, NSEG], f32, tag="sg2")
                nc.vector.tensor_add(sg2[:], sgtT[:, 64 * j:64 * (j + 1)], lnc[:])
                mx = sm.tile([128, 1], f32, tag="mx")
                nc.vector.reduce_max(mx[:], sg2[:], axis=Xax)
                negmx = sm.tile([128, 1], f32, tag="negmx")
                nc.vector.tensor_scalar_mul(negmx[:], mx[:], -1.0)
                u = sm.tile([128, NSEG], f16, tag="u")
                dsum = sm.tile([128, 1], f32, tag="dsum")
                nc.scalar.activation(u[:], sg2[:], Exp, bias=negmx[:],
                                     accum_out=dsum[:])
                rd = sm.tile([128, 1], f32, tag="rd")
                nc.vector.reciprocal(rd[:], dsum[:])
                urn = sm.tile([128, NSEG], f16, tag="urn", bufs=5)
                nc.vector.tensor_scalar_mul(urn[:], u[:], rd[:])
                nc.scalar.dma_start(urn_o.ap()[b, 128 * j:128 * (j + 1), :], urn[:])
                st[f"urn{j}"] = urn

        def emit_urt(b):
            # urT for the K=65 apply matmuls, all four q-tiles at once;
            # emitted a couple of qlo blocks late so the PE never waits on
            # the softmax chain.
            st = state[b]
            urt = st["urt"]
            pu = ps_t.tile([128, 1024], f16, tag="tp")
            for j in range(QT):
                nc.tensor.transpose(pu[0:64, 128 * j:128 * (j + 1)],
                                    st[f"urn{j}"][:], identh[:])
            nc.vector.tensor_copy(urt[0:64, :], pu[0:64, 0:512])

        def emit_p(b):
            st = state[b]
            csumt = st["csumt8"]
            # P_aug[0:64] = Cavg @ W1 ; row 64 = bias
            p_lo = ps64.tile([64, 512], f32, tag="a64")
            p_hi = ps64.tile([64, 512], f32, tag="a64")
            for d in range(DT):
                ct_d = csumt[:, NSEG * d:NSEG * (d + 1)]
                nc.tensor.matmul(p_lo[:], ct_d, w1_all[:, D * d:D * d + 512],
                                 start=(d == 0), stop=(d == DT - 1))
                nc.tensor.matmul(p_hi[:], ct_d, w1_all[:, D * d + 512:D * (d + 1)],
                                 start=(d == 0), stop=(d == DT - 1))
            paug = front.tile([65, D], f16, tag="paug")
            nc.vector.tensor_scalar_mul(paug[0:64, 0:512], p_lo[:], 1.0 / 4096.0)
            nc.vector.tensor_scalar_mul(paug[0:64, 512:1024], p_hi[:], 1.0 / 4096.0)
            nc.vector.tensor_copy(paug[64:65, :], bias_sb[:])
            st["paug"] = paug

        def emit_qlo(b, j):
            st = state[b]
            ott = st["ott"]
            o_lo = ps_lo.tile([128, 512], f32, tag="po_lo")
            for d in range(DT):
                otd = ott[:, Q * d + 128 * j:Q * d + 128 * (j + 1)]
                nc.tensor.matmul(o_lo[:], otd, w2_all[:, D * d:D * d + 512],
                                 start=(d == 0), stop=False)
            st[f"q{j}"] = o_lo

        def emit_qhi(b, j):
            st = state[b]
            ott = st["ott"]
            o_hi = ps_hi.tile([128, 512], f32, tag="po_hi")
            for d in range(DT):
                otd = ott[:, Q * d + 128 * j:Q * d + 128 * (j + 1)]
                nc.tensor.matmul(o_hi[:], otd, w2_all[:, D * d + 512:D * (d + 1)],
                                 start=(d == 0), stop=False)
            st[f"qh{j}"] = o_hi

        def emit_aplo(b, j):
            st = state[b]
            o_lo = st[f"q{j}"]
            urt, paug = st["urt"], st["paug"]
            ua = urt[:, 128 * j:128 * (j + 1)]
            nc.tensor.matmul(o_lo[:], ua, paug[:, 0:512], start=False, stop=True)
            ost = stage.tile([128, 512], f16, tag="ostl")
            nc.scalar.activation(ost[:], o_lo[:], Tanh)
            nc.scalar.dma_start(out_o.ap()[b, 128 * j:128 * (j + 1), 0:512], ost[:])

        def emit_aphi(b, j):
            st = state[b]
            o_hi = st[f"qh{j}"]
            urt, paug = st["urt"], st["paug"]
            ua = urt[:, 128 * j:128 * (j + 1)]
            nc.tensor.matmul(o_hi[:], ua, paug[:, 512:1024], start=False, stop=True)
            ost = stage.tile([128, 512], f16, tag="osth")
            nc.scalar.activation(ost[:], o_hi[:], Tanh)
            nc.scalar.dma_start(out_o.ap()[b, 128 * j:128 * (j + 1), 512:1024], ost[:])

        # ---- emission ----
        # sync ring: batch-0 ctx (256KB chunks for steady HAM-safe cadence) +
        # O^T, then w2-hi, then the batch-1 bulk.  scalar ring (slow at
        # bulk): early batch-1 ctx head.
        cv0, cs0 = emit_loads(0, nc.sync)
        for t in range(ST):
            nc.sync.dma_start(cv0[:, t:t + 1, :], cs0[:, t:t + 1, :])
        emit_load_ott(0, nc.sync)
        nc.sync.dma_start(w2v[:, 0:4, 512:1024], w2s[:, 0:4, 512:1024])
        nc.sync.dma_start(w2v[:, 4:8, 512:1024], w2s[:, 4:8, 512:1024])
        cv1, cs1 = emit_loads(1, nc.scalar)
        for t in range(6):
            nc.scalar.dma_start(cv1[:, t:t + 1, :], cs1[:, t:t + 1, :])
        for t in range(6, 8):
            nc.sync.dma_start(cv1[:, t:t + 1, :], cs1[:, t:t + 1, :])
        emit_load_ott(1, nc.sync)

        emit_warmup()
        emit_csum(0)
        emit_segavg(0)
        emit_rewarm(24)
        emit_p(0)
        emit_qlo(0, 0)
        emit_qlo(0, 1)
        emit_urt(0)
        emit_aplo(0, 0)
        emit_qlo(0, 2)
        emit_aplo(0, 1)
        emit_qlo(0, 3)
        emit_aplo(0, 2)
        emit_aplo(0, 3)
        emit_qhi(0, 0)
        emit_csum(1, 0, 2)
        emit_aphi(0, 0)
        emit_qhi(0, 1)
        emit_csum(1, 2, 4)
        emit_aphi(0, 1)
        emit_qhi(0, 2)
        emit_csum(1, 4, 6)
        emit_aphi(0, 2)
        emit_qhi(0, 3)
        emit_csum(1, 6, 8)
        emit_aphi(0, 3)
        emit_p(1)
        emit_segavg(1)
        emit_qlo(1, 0)
        emit_qlo(1, 1)
        emit_urt(1)
        emit_aplo(1, 0)
        emit_qlo(1, 2)
        emit_aplo(1, 1)
        emit_qlo(1, 3)
        emit_aplo(1, 2)
        emit_aplo(1, 3)
        emit_qhi(1, 0)
        emit_aphi(1, 0)
        emit_qhi(1, 1)
        emit_aphi(1, 1)
        emit_qhi(1, 2)
        emit_aphi(1, 2)
        emit_qhi(1, 3)
        emit_aphi(1, 3)

    nc.compile()
    return nc


def _host_prep(output, context, W_weight, W_bias, segment_ids):
    """Shard over batch; fp16 conversion + index/layout prep (no reductions)."""
    import concourse.mybir as mybir
    np_f8 = mybir.dt.np(mybir.dt.float8e4)
    wt = W_weight.T.astype(np.float16)                       # [2D, D]
    w1 = np.ascontiguousarray((wt[:D].astype(np.float32) * 256.0).astype(np_f8))
    w2 = np.ascontiguousarray(wt[D:])
    biasr = np.ascontiguousarray(W_bias.astype(np.float16)[None, :])
    identh = np.eye(128, dtype=np.float16)

    in_maps, aligns = [], []
    for c in range(NCORES):
        lo = c * BPC
        ohis, lncs, invcs = [], [], []
        for b in range(BPC):
            ids = segment_ids[lo + b].astype(np.int64)       # [S]
            oh = (ids[:, None] == np.arange(NSEG)[None, :]).astype(np.float32)
            cnt = oh.sum(axis=0)                             # [NSEG]
            invc = 1.0 / np.maximum(cnt, 1.0)
            ohi = (oh * invc[None, :]).astype(np.float16)    # [S, NSEG]
            ohis.append(np.ascontiguousarray(
                ohi.reshape(ST, 128, NSEG).transpose(1, 0, 2).reshape(128, ST * NSEG)))
            lnrow = np.where(cnt > 0, np.log(np.maximum(cnt, 1.0)), -1e30)
            lncs.append(np.ascontiguousarray(np.broadcast_to(
                lnrow.astype(np.float32)[None, :], (128, NSEG))))
            invcs.append(invc)
        in_maps.append({
            "ctx_in": np.ascontiguousarray(context[lo:lo + BPC].astype(np.float16)),
            "ott_in": np.ascontiguousarray(
                output[lo:lo + BPC].astype(np.float16).transpose(0, 2, 1)),
            "w1_in": w1, "w2_in": w2, "bias_in": biasr, "identh_in": identh,
            "ohi_in": np.stack(ohis), "lnc_in": np.stack(lncs),
        })
        aligns.append(invcs)
    return in_maps, aligns


def _run(inputs, trace=False, tmpdir=None):
    from concourse.bass_utils import run_bass_kernel_spmd
    if "nc" not in _CACHE:
        _CACHE["nc"] = _build_nc()
    nc = _CACHE["nc"]
    in_maps, invcs = _host_prep(**inputs)
    kw = {}
    if trace:
        kw = {"trace": True, "tmpdir": tmpdir}
    res = run_bass_kernel_spmd(nc, in_maps, core_ids=list(range(NCORES)), **kw)
    out = np.concatenate(
        [res.results[c]["out_o"].astype(np.float32) for c in range(NCORES)], axis=0)
    # align[q, s] = urn[q, seg(s)] * invc[seg(s)]  — host-side gather/unshard
    seg = inputs["segment_ids"]
    align = np.empty((B, Q, S), dtype=np.float32)
    for c in range(NCORES):
        for b in range(BPC):
            gb = c * BPC + b
            urn = res.results[c]["urn_o"][b].astype(np.float32)   # [Q, NSEG]
            scaled = urn * invcs[c][b][None, :].astype(np.float32)
            align[gb] = scaled[:, seg[gb].astype(np.int64)]
    return (out, align), res


def kernel(output, context, W_weight, W_bias, segment_ids):
    # Force host numpy up front: if the caller hands us jax arrays, numpy
    # ops would otherwise dispatch to the accelerator backend.
    (out, align), _ = _run(dict(
        output=np.asarray(output, dtype=np.float32),
        context=np.asarray(context, dtype=np.float32),
        W_weight=np.asarray(W_weight, dtype=np.float32),
        W_bias=np.asarray(W_bias, dtype=np.float32),
        segment_ids=np.asarray(segment_ids, dtype=np.int32)))
    return out, align


# revision 70
# speedup vs baseline: 1.0177x; 1.0177x over previous
"""Trainium2 Bass kernel for nn_Attn_30820685316537 (segment_reduce attention).

Reference computation (per batch b):
    score = output @ context^T                     [Q, S]
    avg   = per-segment mean of score over S, broadcast back
    align = softmax(avg, axis=S)                   [Q, S]
    ac    = align @ context                        [Q, D]
    out   = tanh(concat(ac, output) @ W^T + bias)  [Q, D]
    returns (out, align)

Everything factors through rank-64 segment space (avg is constant within each
contiguous segment).  With Cavg[n, d] = (1/cnt_n) * sum_{s in seg n} C[s, d]:
    segavg[q, n] = O[q, :] . Cavg[n, :]
    u[q, n]      = exp(segavg + ln cnt_n - max)        (cnt-weighted softmax)
    urn[q, n]    = u / sum_n u                         (per-segment align mass)
    align[q, s]  = urn[q, seg(s)] / cnt_{seg(s)}       (host-side gather)
    ac[q, :]     = urn @ Cavg
    out          = tanh(urn @ (Cavg @ W1) + O @ W2 + bias)
where W^T = [W1; W2].  P = Cavg @ W1 is a [64, D] matrix, so the output GEMM
is O @ W2 (K=1024) plus a K=65 rank-64 correction (bias folded in as P's 65th
row) instead of the K=2048 concat GEMM.

Device schedule per batch:
  front: Cavg  = ohi^T @ ctx (one-hot matmul, invc pre-folded on host),
         CavgT = 8 PE transposes, segavgT = CavgT^T @ O^T as 8 wide (N=512)
         matmuls, 4 transposes back to [q, n], then all four q-tiles'
         softmaxes run up front on DVE/ACT (decoupled from the GEMM stream),
         P = CavgT^T @ W1 in fp8 (x4096 scaling keeps fp8 normals).
  qloop (per 128-row q-tile): O@W2 lo columns, then hi columns (the W2
         column-halves stream from HBM in that order); urn is transposed
         (batched, one PSUM tile) and applied (K=65) into the same PSUM
         accumulators; tanh + DMA out per half.
Two batches per core are software-pipelined; GEMMs fp16 (fp32 PSUM
accumulation), P-path fp8.  align is reconstructed on the host by a pure
gather of the device-computed urn masses.  DMA rings are byte-balanced and
ordered by need-time (ctx0/W early, batch-1 inputs behind batch-0 on the
same queues); warm-up/keep-alive matmuls hold the PE's HAM clock-gate at
full rate through the input-paced ramp.

Sharding: data-parallel over batch B=16 across 8 NeuronCores; W replicated.
"""
import numpy as np
from contextlib import ExitStack

B, Q, S, D = 16, 512, 1024, 1024
NSEG = 64
NCORES = 8
BPC = B // NCORES          # batches per core
QT = Q // 128              # 4 q-tiles
ST = S // 128              # 8 s-chunks
DT = D // 128              # 8 d-chunks

_CACHE = {}


def _build_nc():
    import concourse.bacc as bacc
    import concourse.tile as tile
    import concourse.mybir as mybir

    f32 = mybir.dt.float32
    f16 = mybir.dt.float16
    f8 = mybir.dt.float8e4

    nc = bacc.Bacc("TRN2", target_bir_lowering=False, debug=False,
                   enable_asserts=False, num_devices=NCORES)

    ctx_in = nc.dram_tensor("ctx_in", [BPC, S, D], f16, kind="ExternalInput")
    ott_in = nc.dram_tensor("ott_in", [BPC, D, Q], f16, kind="ExternalInput")   # O^T
    w1_in = nc.dram_tensor("w1_in", [D, D], f8, kind="ExternalInput")           # WT[:D]
    w2_in = nc.dram_tensor("w2_in", [D, D], f16, kind="ExternalInput")          # WT[D:]
    ohi_in = nc.dram_tensor("ohi_in", [BPC, 128, ST * NSEG], f16, kind="ExternalInput")
    lnc_in = nc.dram_tensor("lnc_in", [BPC, 128, NSEG], f32, kind="ExternalInput")
    bias_in = nc.dram_tensor("bias_in", [1, D], f16, kind="ExternalInput")
    identh_in = nc.dram_tensor("identh_in", [128, 128], f16, kind="ExternalInput")

    out_o = nc.dram_tensor("out_o", [BPC, Q, D], f16, kind="ExternalOutput")
    urn_o = nc.dram_tensor("urn_o", [BPC, Q, NSEG], f16, kind="ExternalOutput")

    Exp = mybir.ActivationFunctionType.Exp
    Tanh = mybir.ActivationFunctionType.Tanh

    with tile.TileContext(nc) as tc, ExitStack() as ectx:
        consts = ectx.enter_context(tc.tile_pool(name="consts", bufs=1))
        inp = ectx.enter_context(tc.tile_pool(name="inp", bufs=2))
        front = ectx.enter_context(tc.tile_pool(name="front", bufs=2))
        sm = ectx.enter_context(tc.tile_pool(name="sm", bufs=3))
        stage = ectx.enter_context(tc.tile_pool(name="stage", bufs=2))

        # PSUM: exactly 8 banks (2 + 1 + 2 + 2 + 1 junk).
        ps64 = ectx.enter_context(tc.tile_pool(name="ps64", bufs=2, space="PSUM"))
        ps_t = ectx.enter_context(tc.tile_pool(name="ps_t", bufs=1, space="PSUM"))
        ps_lo = ectx.enter_context(tc.tile_pool(name="ps_lo", bufs=2, space="PSUM"))
        ps_hi = ectx.enter_context(tc.tile_pool(name="ps_hi", bufs=2, space="PSUM"))
        ps_j = ectx.enter_context(tc.tile_pool(name="ps_j", bufs=1, space="PSUM"))

        # ---- const loads (gpsimd ring: identh first, then W1, then W2-lo) ----
        identh = consts.tile([128, 128], f16, tag="identh")
        nc.gpsimd.dma_start(identh[:], identh_in.ap())
        bias_sb = consts.tile([1, D], f16, tag="bias")
        nc.gpsimd.dma_start(bias_sb[:], bias_in.ap())

        w1_all = consts.tile([128, DT * D], f8, tag="w1")      # [p, (d f)] fp8
        w1v = w1_all[:].rearrange("p (c f) -> p c f", f=D)
        w1s = w1_in.ap().rearrange("(c p) f -> p c f", p=128)
        for t in range(2):
            nc.gpsimd.dma_start(w1v[:, 4 * t:4 * (t + 1), :], w1s[:, 4 * t:4 * (t + 1), :])

        w2_all = consts.tile([128, DT * D], f16, tag="w2")
        w2v = w2_all[:].rearrange("p (c f) -> p c f", f=D)
        w2s = w2_in.ap().rearrange("(c p) f -> p c f", p=128)
        # lo columns of every chunk first (feeds the qlo pass ~6µs before
        # the qhi pass needs the hi columns; w2-hi rides the sync ring).
        nc.gpsimd.dma_start(w2v[:, 0:4, 0:512], w2s[:, 0:4, 0:512])
        nc.gpsimd.dma_start(w2v[:, 4:8, 0:512], w2s[:, 4:8, 0:512])

        Xax = mybir.AxisListType.X

        state = [dict() for _ in range(BPC)]

        def emit_loads(b, eng):
            st = state[b]
            ohi = inp.tile([128, ST * NSEG], f16, tag="ohi")
            eng.dma_start(ohi[:], ohi_in.ap()[b])
            lnc = inp.tile([128, NSEG], f32, tag="lnc")
            eng.dma_start(lnc[:], lnc_in.ap()[b])
            ctx_all = inp.tile([128, ST * D], f16, tag="ctx")   # [p, (i d)]
            cv = ctx_all[:].rearrange("p (c d) -> p c d", d=D)
            cs = ctx_in.ap()[b].rearrange("(c p) d -> p c d", p=128)
            st["ohi"], st["lnc"], st["ctx"] = ohi, lnc, ctx_all
            return cv, cs

        def emit_load_ott(b, eng):
            st = state[b]
            ott_all = inp.tile([128, DT * Q], f16, tag="ott")   # [p, (d q)]
            ov = ott_all[:].rearrange("p (c q) -> p c q", q=Q)
            os_ = ott_in.ap()[b].rearrange("(c p) q -> p c q", p=128)
            eng.dma_start(ov[:, 0:4, :], os_[:, 0:4, :])
            eng.dma_start(ov[:, 4:8, :], os_[:, 4:8, :])
            st["ott"] = ott_all

        junk = [None]

        def emit_warmup():
            # Dense PE work during the initial DMA wait so HAM un-throttles
            # before the real stream starts (identh arrives in the first µs).
            junk[0] = ps_j.tile([128, 512], f32, tag="junk", name="junkpw")
            pw = junk[0]
            for r in range(32):
                nc.tensor.matmul(pw[:, 0:128], identh[:], identh[:],
                                 start=(r == 0), stop=(r == 31))

        def emit_rewarm(n):
            # Keep-alive matmuls emitted just before a known input-wait point:
            # they execute during the DMA wait and flip HAM back to full clock
            # before the dense GEMM stream starts.
            pw = junk[0]
            for r in range(n):
                nc.tensor.matmul(pw[:, 0:128], identh[:], identh[:],
                                 start=True, stop=True)

        def emit_csum_mm(b, i):
            st = state[b]
            ohi, ctx_all = st["ohi"], st["ctx"]
            if i == 0:
                st["cs_lo"] = ps64.tile([64, 512], f32, tag="a64", name="cs_lo")
                st["cs_hi"] = ps64.tile([64, 512], f32, tag="a64", name="cs_hi")
            oh_i = ohi[:, NSEG * i:NSEG * (i + 1)]
            nc.tensor.matmul(st["cs_lo"][:], oh_i, ctx_all[:, D * i:D * i + 512],
                             start=(i == 0), stop=(i == ST - 1))
            nc.tensor.matmul(st["cs_hi"][:], oh_i, ctx_all[:, D * i + 512:D * (i + 1)],
                             start=(i == 0), stop=(i == ST - 1))

        def emit_csum(b, lo=0, hi=ST):
            st = state[b]
            for i in range(lo, hi):
                if b == 0:
                    emit_rewarm(6)   # fill the ctx-arrival pacing gap
                emit_csum_mm(b, i)
            if hi < ST:
                return
            cs_lo, cs_hi = st["cs_lo"], st["cs_hi"]
            csum = front.tile([64, D], f16, tag="csum")
            nc.vector.tensor_copy(csum[:, 0:512], cs_lo[:])
            nc.vector.tensor_copy(csum[:, 512:1024], cs_hi[:])

            # CavgT packed [128, (d n)] via 8 PE transposes
            csumt = front.tile([128, DT * NSEG], f16, tag="csumt")
            pt = ps_t.tile([128, 1024], f16, tag="tp")
            pt2 = ps_t.tile([128, 1024], f16, tag="tp")
            for d in range(DT):
                po = pt if d < 4 else pt2
                nc.tensor.transpose(po[:, 64 * (d % 4):64 * (d % 4 + 1)],
                                    csum[0:64, 128 * d:128 * (d + 1)],
                                    identh[0:64, 0:64])
            nc.vector.tensor_copy(csumt[:, 0:256], pt[:, 0:256])
            nc.vector.tensor_copy(csumt[:, 256:512], pt2[:, 0:256])
            st["csumt"] = csumt
            csumt8 = front.tile([128, DT * NSEG], f8, tag="csumt8")
            nc.vector.tensor_scalar_mul(csumt8[:, 0:256], pt[:, 0:256], 16.0)
            nc.vector.tensor_scalar_mul(csumt8[:, 256:512], pt2[:, 0:256], 16.0)
            st["csumt8"] = csumt8

            urt = front.tile([65, Q], f16, tag="urt")
            nc.vector.memset(urt[64:65, :], 1.0)
            st["urt"] = urt

        def emit_segavg(b):
            # segavgT[n, q] = Cavg @ O^T as 8 wide matmuls, then transpose to
            # [q, n] and run all four q-tiles' softmaxes up front (decoupled
            # from the O@W2 stream).
            st = state[b]
            csumt, ott, lnc = st["csumt"], st["ott"], st["lnc"]
            sgt = ps64.tile([64, Q], f32, tag="a64")
            for d in range(DT):
                nc.tensor.matmul(sgt[:], csumt[:, NSEG * d:NSEG * (d + 1)],
                                 ott[:, Q * d:Q * (d + 1)],
                                 start=(d == 0), stop=(d == DT - 1))
            sgt_sb = front.tile([64, Q], f16, tag="sgt")
            nc.vector.tensor_copy(sgt_sb[:], sgt[:])
            sgtT = ps_t.tile([128, 1024], f16, tag="tp")
            for j in range(QT):
                nc.tensor.transpose(sgtT[:, 64 * j:64 * (j + 1)],
                                    sgt_sb[0:64, 128 * j:128 * (j + 1)],
                                    identh[0:64, 0:64])
            for j in range(QT):
                sg2 = sm.tile([128, NSEG], f32, tag="sg2")
                nc.vector.tensor_add(sg2[:], sgtT[:, 64 * j:64 * (j + 1)], lnc[:])
                mx = sm.tile([128, 1], f32, tag="mx")
                nc.vector.reduce_max(mx[:], sg2[:], axis=Xax)
                negmx = sm.tile([128, 1], f32, tag="negmx")
                nc.vector.tensor_scalar_mul(negmx[:], mx[:], -1.0)
                u = sm.tile([128, NSEG], f16, tag="u")
                dsum = sm.tile([128, 1], f32, tag="dsum")
                nc.scalar.activation(u[:], sg2[:], Exp, bias=negmx[:],
                                     accum_out=dsum[:])
                rd = sm.tile([128, 1], f32, tag="rd")
                nc.vector.reciprocal(rd[:], dsum[:])
                urn = sm.tile([128, NSEG], f16, tag="urn", bufs=5)
                nc.vector.tensor_scalar_mul(urn[:], u[:], rd[:])
                nc.scalar.dma_start(urn_o.ap()[b, 128 * j:128 * (j + 1), :], urn[:])
                st[f"urn{j}"] = urn

        def emit_urt(b):
            # urT for the K=65 apply matmuls, all four q-tiles at once;
            # emitted a couple of qlo blocks late so the PE never waits on
            # the softmax chain.
            st = state[b]
            urt = st["urt"]
            pu = ps_t.tile([128, 1024], f16, tag="tp")
            for j in range(QT):
                nc.tensor.transpose(pu[0:64, 128 * j:128 * (j + 1)],
                                    st[f"urn{j}"][:], identh[:])
            nc.vector.tensor_copy(urt[0:64, :], pu[0:64, 0:512])

        def emit_p(b):
            st = state[b]
            csumt = st["csumt8"]
            # P_aug[0:64] = Cavg @ W1 ; row 64 = bias
            p_lo = ps64.tile([64, 512], f32, tag="a64")
            p_hi = ps64.tile([64, 512], f32, tag="a64")
            for d in range(DT):
                ct_d = csumt[:, NSEG * d:NSEG * (d + 1)]
                nc.tensor.matmul(p_lo[:], ct_d, w1_all[:, D * d:D * d + 512],
                                 start=(d == 0), stop=(d == DT - 1))
                nc.tensor.matmul(p_hi[:], ct_d, w1_all[:, D * d + 512:D * (d + 1)],
                                 start=(d == 0), stop=(d == DT - 1))
            paug = front.tile([65, D], f16, tag="paug")
            nc.vector.tensor_scalar_mul(paug[0:64, 0:512], p_lo[:], 1.0 / 4096.0)
            nc.vector.tensor_scalar_mul(paug[0:64, 512:1024], p_hi[:], 1.0 / 4096.0)
            nc.vector.tensor_copy(paug[64:65, :], bias_sb[:])
            st["paug"] = paug

        def emit_qlo(b, j):
            st = state[b]
            ott = st["ott"]
            o_lo = ps_lo.tile([128, 512], f32, tag="po_lo")
            for d in range(DT):
                otd = ott[:, Q * d + 128 * j:Q * d + 128 * (j + 1)]
                nc.tensor.matmul(o_lo[:], otd, w2_all[:, D * d:D * d + 512],
                                 start=(d == 0), stop=False)
            st[f"q{j}"] = o_lo

        def emit_qhi(b, j):
            st = state[b]
            ott = st["ott"]
            o_hi = ps_hi.tile([128, 512], f32, tag="po_hi")
            for d in range(DT):
                otd = ott[:, Q * d + 128 * j:Q * d + 128 * (j + 1)]
                nc.tensor.matmul(o_hi[:], otd, w2_all[:, D * d + 512:D * (d + 1)],
                                 start=(d == 0), stop=False)
            st[f"qh{j}"] = o_hi

        def emit_aplo(b, j):
            st = state[b]
            o_lo = st[f"q{j}"]
            urt, paug = st["urt"], st["paug"]
            ua = urt[:, 128 * j:128 * (j + 1)]
            nc.tensor.matmul(o_lo[:], ua, paug[:, 0:512], start=False, stop=True)
            ost = stage.tile([128, 512], f16, tag="ostl")
            nc.scalar.activation(ost[:], o_lo[:], Tanh)
            nc.scalar.dma_start(out_o.ap()[b, 128 * j:128 * (j + 1), 0:512], ost[:])

        def emit_aphi(b, j):
            st = state[b]
            o_hi = st[f"qh{j}"]
            urt, paug = st["urt"], st["paug"]
            ua = urt[:, 128 * j:128 * (j + 1)]
            nc.tensor.matmul(o_hi[:], ua, paug[:, 512:1024], start=False, stop=True)
            ost = stage.tile([128, 512], f16, tag="osth")
            nc.scalar.activation(ost[:], o_hi[:], Tanh)
            nc.scalar.dma_start(out_o.ap()[b, 128 * j:128 * (j + 1), 512:1024], ost[:])

        # ---- emission ----
        # sync ring: batch-0 ctx (256KB chunks for steady HAM-safe cadence) +
        # O^T, then w2-hi, then the batch-1 bulk.  scalar ring (slow at
        # bulk): early batch-1 ctx head.
        cv0, cs0 = emit_loads(0, nc.sync)
        for t in range(ST):
            nc.sync.dma_start(cv0[:, t:t + 1, :], cs0[:, t:t + 1, :])
        emit_load_ott(0, nc.sync)
        nc.sync.dma_start(w2v[:, 0:4, 512:1024], w2s[:, 0:4, 512:1024])
        nc.sync.dma_start(w2v[:, 4:8, 512:1024], w2s[:, 4:8, 512:1024])
        cv1, cs1 = emit_loads(1, nc.scalar)
        for t in range(6):
            nc.scalar.dma_start(cv1[:, t:t + 1, :], cs1[:, t:t + 1, :])
        for t in range(6, 8):
            nc.sync.dma_start(cv1[:, t:t + 1, :], cs1[:, t:t + 1, :])
        emit_load_ott(1, nc.sync)

        emit_warmup()
        emit_csum(0)
        emit_segavg(0)
        emit_rewarm(40)
        emit_qlo(0, 0)
        emit_qlo(0, 1)
        emit_urt(0)
        emit_p(0)
        emit_aplo(0, 0)
        emit_qlo(0, 2)
        emit_aplo(0, 1)
        emit_qlo(0, 3)
        emit_aplo(0, 2)
        emit_aplo(0, 3)
        emit_qhi(0, 0)
        emit_csum(1, 0, 2)
        emit_aphi(0, 0)
        emit_qhi(0, 1)
        emit_csum(1, 2, 4)
        emit_aphi(0, 1)
        emit_qhi(0, 2)
        emit_csum(1, 4, 6)
        emit_aphi(0, 2)
        emit_qhi(0, 3)
        emit_csum(1, 6, 8)
        emit_aphi(0, 3)
        emit_segavg(1)
        emit_p(1)
        emit_qlo(1, 0)
        emit_qlo(1, 1)
        emit_urt(1)
        emit_aplo(1, 0)
        emit_qlo(1, 2)
        emit_aplo(1, 1)
        emit_qlo(1, 3)
        emit_aplo(1, 2)
        emit_aplo(1, 3)
        emit_qhi(1, 0)
        emit_aphi(1, 0)
        emit_qhi(1, 1)
        emit_aphi(1, 1)
        emit_qhi(1, 2)
        emit_aphi(1, 2)
        emit_qhi(1, 3)
        emit_aphi(1, 3)

    nc.compile()
    return nc


def _host_prep(output, context, W_weight, W_bias, segment_ids):
    """Shard over batch; fp16 conversion + index/layout prep (no reductions)."""
    import concourse.mybir as mybir
    np_f8 = mybir.dt.np(mybir.dt.float8e4)
    wt = W_weight.T.astype(np.float16)                       # [2D, D]
    w1 = np.ascontiguousarray((wt[:D].astype(np.float32) * 256.0).astype(np_f8))
    w2 = np.ascontiguousarray(wt[D:])
    biasr = np.ascontiguousarray(W_bias.astype(np.float16)[None, :])
    identh = np.eye(128, dtype=np.float16)

    in_maps, aligns = [], []
    for c in range(NCORES):
        lo = c * BPC
        ohis, lncs, invcs = [], [], []
        for b in range(BPC):
            ids = segment_ids[lo + b].astype(np.int64)       # [S]
            oh = (ids[:, None] == np.arange(NSEG)[None, :]).astype(np.float32)
            cnt = oh.sum(axis=0)                             # [NSEG]
            invc = 1.0 / np.maximum(cnt, 1.0)
            ohi = (oh * invc[None, :]).astype(np.float16)    # [S, NSEG]
            ohis.append(np.ascontiguousarray(
                ohi.reshape(ST, 128, NSEG).transpose(1, 0, 2).reshape(128, ST * NSEG)))
            lnrow = np.where(cnt > 0, np.log(np.maximum(cnt, 1.0)), -1e30)
            lncs.append(np.ascontiguousarray(np.broadcast_to(
                lnrow.astype(np.float32)[None, :], (128, NSEG))))
            invcs.append(invc)
        in_maps.append({
            "ctx_in": np.ascontiguousarray(context[lo:lo + BPC].astype(np.float16)),
            "ott_in": np.ascontiguousarray(
                output[lo:lo + BPC].astype(np.float16).transpose(0, 2, 1)),
            "w1_in": w1, "w2_in": w2, "bias_in": biasr, "identh_in": identh,
            "ohi_in": np.stack(ohis), "lnc_in": np.stack(lncs),
        })
        aligns.append(invcs)
    return in_maps, aligns


def _run(inputs, trace=False, tmpdir=None):
    from concourse.bass_utils import run_bass_kernel_spmd
    if "nc" not in _CACHE:
        _CACHE["nc"] = _build_nc()
    nc = _CACHE["nc"]
    in_maps, invcs = _host_prep(**inputs)
    kw = {}
    if trace:
        kw = {"trace": True, "tmpdir": tmpdir}
    res = run_bass_kernel_spmd(nc, in_maps, core_ids=list(range(NCORES)), **kw)
    out = np.concatenate(
        [res.results[c]["out_o"].astype(np.float32) for c in range(NCORES)], axis=0)
    # align[q, s] = urn[q, seg(s)] * invc[seg(s)]  — host-side gather/unshard
    seg = inputs["segment_ids"]
    align = np.empty((B, Q, S), dtype=np.float32)
    for c in range(NCORES):
        for b in range(BPC):
            gb = c * BPC + b
            urn = res.results[c]["urn_o"][b].astype(np.float32)   # [Q, NSEG]
            scaled = urn * invcs[c][b][None, :].astype(np.float32)
            align[gb] = scaled[:, seg[gb].astype(np.int64)]
    return (out, align), res


def kernel(output, context, W_weight, W_bias, segment_ids):
    # Force host numpy up front: if the caller hands us jax arrays, numpy
    # ops would otherwise dispatch to the accelerator backend.
    (out, align), _ = _run(dict(
        output=np.asarray(output, dtype=np.float32),
        context=np.asarray(context, dtype=np.float32),
        W_weight=np.asarray(W_weight, dtype=np.float32),
        W_bias=np.asarray(W_bias, dtype=np.float32),
        segment_ids=np.asarray(segment_ids, dtype=np.int32)))
    return out, align


# revision 71
# speedup vs baseline: 1.0541x; 1.0358x over previous
"""Trainium2 Bass kernel for nn_Attn_30820685316537 (segment_reduce attention).

Reference computation (per batch b):
    score = output @ context^T                     [Q, S]
    avg   = per-segment mean of score over S, broadcast back
    align = softmax(avg, axis=S)                   [Q, S]
    ac    = align @ context                        [Q, D]
    out   = tanh(concat(ac, output) @ W^T + bias)  [Q, D]
    returns (out, align)

Everything factors through rank-64 segment space (avg is constant within each
contiguous segment).  With Cavg[n, d] = (1/cnt_n) * sum_{s in seg n} C[s, d]:
    segavg[q, n] = O[q, :] . Cavg[n, :]
    u[q, n]      = exp(segavg + ln cnt_n - max)        (cnt-weighted softmax)
    urn[q, n]    = u / sum_n u                         (per-segment align mass)
    align[q, s]  = urn[q, seg(s)] / cnt_{seg(s)}       (host-side gather)
    ac[q, :]     = urn @ Cavg
    out          = tanh(urn @ (Cavg @ W1) + O @ W2 + bias)
where W^T = [W1; W2].  P = Cavg @ W1 is a [64, D] matrix, so the output GEMM
is O @ W2 (K=1024) plus a K=65 rank-64 correction (bias folded in as P's 65th
row) instead of the K=2048 concat GEMM.

Device schedule per batch:
  front: Cavg  = ohi^T @ ctx (one-hot matmul, invc pre-folded on host),
         CavgT = 8 PE transposes, segavgT = CavgT^T @ O^T as 8 wide (N=512)
         matmuls, 4 transposes back to [q, n], then all four q-tiles'
         softmaxes run up front on DVE/ACT (decoupled from the GEMM stream),
         P = CavgT^T @ W1 in fp8 (x4096 scaling keeps fp8 normals).
  qloop (per 128-row q-tile): O@W2 lo columns, then hi columns (the W2
         column-halves stream from HBM in that order); urn is transposed
         (batched, one PSUM tile) and applied (K=65) into the same PSUM
         accumulators; tanh + DMA out per half.
Two batches per core are software-pipelined; GEMMs fp16 (fp32 PSUM
accumulation), P-path fp8.  align is reconstructed on the host by a pure
gather of the device-computed urn masses.  DMA rings are byte-balanced and
ordered by need-time (ctx0/W early, batch-1 inputs behind batch-0 on the
same queues); warm-up/keep-alive matmuls hold the PE's HAM clock-gate at
full rate through the input-paced ramp.

Sharding: data-parallel over batch B=16 across 8 NeuronCores; W replicated.
"""
import numpy as np
from contextlib import ExitStack

B, Q, S, D = 16, 512, 1024, 1024
NSEG = 64
NCORES = 8
BPC = B // NCORES          # batches per core
QT = Q // 128              # 4 q-tiles
ST = S // 128              # 8 s-chunks
DT = D // 128              # 8 d-chunks

_CACHE = {}


def _build_nc():
    import concourse.bacc as bacc
    import concourse.tile as tile
    import concourse.mybir as mybir

    f32 = mybir.dt.float32
    f16 = mybir.dt.float16
    f8 = mybir.dt.float8e4

    nc = bacc.Bacc("TRN2", target_bir_lowering=False, debug=False,
                   enable_asserts=False, num_devices=NCORES)

    ctx_in = nc.dram_tensor("ctx_in", [BPC, S, D], f16, kind="ExternalInput")
    ott_in = nc.dram_tensor("ott_in", [BPC, D, Q], f16, kind="ExternalInput")   # O^T
    w1_in = nc.dram_tensor("w1_in", [D, D], f8, kind="ExternalInput")           # WT[:D]
    w2_in = nc.dram_tensor("w2_in", [D, D], f16, kind="ExternalInput")          # WT[D:]
    ohi_in = nc.dram_tensor("ohi_in", [BPC, 128, ST * NSEG], f16, kind="ExternalInput")
    lnc_in = nc.dram_tensor("lnc_in", [BPC, 128, NSEG], f32, kind="ExternalInput")
    bias_in = nc.dram_tensor("bias_in", [1, D], f16, kind="ExternalInput")
    identh_in = nc.dram_tensor("identh_in", [128, 128], f16, kind="ExternalInput")

    out_o = nc.dram_tensor("out_o", [BPC, Q, D], f16, kind="ExternalOutput")
    urn_o = nc.dram_tensor("urn_o", [BPC, Q, NSEG], f16, kind="ExternalOutput")

    Exp = mybir.ActivationFunctionType.Exp
    Tanh = mybir.ActivationFunctionType.Tanh

    with tile.TileContext(nc) as tc, ExitStack() as ectx:
        consts = ectx.enter_context(tc.tile_pool(name="consts", bufs=1))
        inp = ectx.enter_context(tc.tile_pool(name="inp", bufs=2))
        front = ectx.enter_context(tc.tile_pool(name="front", bufs=2))
        sm = ectx.enter_context(tc.tile_pool(name="sm", bufs=3))
        stage = ectx.enter_context(tc.tile_pool(name="stage", bufs=2))

        # PSUM: exactly 8 banks (2 + 1 + 2 + 2 + 1 junk).
        ps64 = ectx.enter_context(tc.tile_pool(name="ps64", bufs=2, space="PSUM"))
        ps_t = ectx.enter_context(tc.tile_pool(name="ps_t", bufs=1, space="PSUM"))
        ps_lo = ectx.enter_context(tc.tile_pool(name="ps_lo", bufs=2, space="PSUM"))
        ps_hi = ectx.enter_context(tc.tile_pool(name="ps_hi", bufs=2, space="PSUM"))
        ps_j = ectx.enter_context(tc.tile_pool(name="ps_j", bufs=1, space="PSUM"))

        # ---- const loads (gpsimd ring: identh first, then W1, then W2-lo) ----
        identh = consts.tile([128, 128], f16, tag="identh")
        nc.gpsimd.dma_start(identh[:], identh_in.ap())
        bias_sb = consts.tile([1, D], f16, tag="bias")
        nc.gpsimd.dma_start(bias_sb[:], bias_in.ap())

        w1_all = consts.tile([128, DT * D], f8, tag="w1")      # [p, (d f)] fp8
        w1v = w1_all[:].rearrange("p (c f) -> p c f", f=D)
        w1s = w1_in.ap().rearrange("(c p) f -> p c f", p=128)
        for t in range(2):
            nc.gpsimd.dma_start(w1v[:, 4 * t:4 * (t + 1), :], w1s[:, 4 * t:4 * (t + 1), :])

        w2_all = consts.tile([128, DT * D], f16, tag="w2")
        w2v = w2_all[:].rearrange("p (c f) -> p c f", f=D)
        w2s = w2_in.ap().rearrange("(c p) f -> p c f", p=128)
        # lo columns of every chunk first (feeds the qlo pass ~6µs before
        # the qhi pass needs the hi columns; w2-hi rides the sync ring).
        nc.gpsimd.dma_start(w2v[:, 0:4, 0:512], w2s[:, 0:4, 0:512])
        nc.gpsimd.dma_start(w2v[:, 4:8, 0:512], w2s[:, 4:8, 0:512])

        Xax = mybir.AxisListType.X

        state = [dict() for _ in range(BPC)]

        def emit_loads(b, eng):
            st = state[b]
            ohi = inp.tile([128, ST * NSEG], f16, tag="ohi")
            eng.dma_start(ohi[:], ohi_in.ap()[b])
            lnc = inp.tile([128, NSEG], f32, tag="lnc")
            eng.dma_start(lnc[:], lnc_in.ap()[b])
            ctx_all = inp.tile([128, ST * D], f16, tag="ctx")   # [p, (i d)]
            cv = ctx_all[:].rearrange("p (c d) -> p c d", d=D)
            cs = ctx_in.ap()[b].rearrange("(c p) d -> p c d", p=128)
            st["ohi"], st["lnc"], st["ctx"] = ohi, lnc, ctx_all
            return cv, cs

        def emit_load_ott(b, eng):
            st = state[b]
            ott_all = inp.tile([128, DT * Q], f16, tag="ott")   # [p, (d q)]
            ov = ott_all[:].rearrange("p (c q) -> p c q", q=Q)
            os_ = ott_in.ap()[b].rearrange("(c p) q -> p c q", p=128)
            eng.dma_start(ov[:, 0:4, :], os_[:, 0:4, :])
            eng.dma_start(ov[:, 4:8, :], os_[:, 4:8, :])
            st["ott"] = ott_all

        junk = [None]

        def emit_warmup():
            # Dense PE work during the initial DMA wait so HAM un-throttles
            # before the real stream starts (identh arrives in the first µs).
            junk[0] = ps_j.tile([128, 512], f32, tag="junk", name="junkpw")
            pw = junk[0]
            for r in range(32):
                nc.tensor.matmul(pw[:, 0:128], identh[:], identh[:],
                                 start=(r == 0), stop=(r == 31))

        def emit_rewarm(n):
            # Keep-alive matmuls emitted just before a known input-wait point:
            # they execute during the DMA wait and flip HAM back to full clock
            # before the dense GEMM stream starts.
            pw = junk[0]
            for r in range(n):
                nc.tensor.matmul(pw[:, 0:128], identh[:], identh[:],
                                 start=True, stop=True)

        def emit_csum_mm(b, i):
            st = state[b]
            ohi, ctx_all = st["ohi"], st["ctx"]
            if i == 0:
                st["cs_lo"] = ps64.tile([64, 512], f32, tag="a64", name="cs_lo")
                st["cs_hi"] = ps64.tile([64, 512], f32, tag="a64", name="cs_hi")
            oh_i = ohi[:, NSEG * i:NSEG * (i + 1)]
            nc.tensor.matmul(st["cs_lo"][:], oh_i, ctx_all[:, D * i:D * i + 512],
                             start=(i == 0), stop=(i == ST - 1))
            nc.tensor.matmul(st["cs_hi"][:], oh_i, ctx_all[:, D * i + 512:D * (i + 1)],
                             start=(i == 0), stop=(i == ST - 1))

        def emit_csum(b, lo=0, hi=ST):
            st = state[b]
            for i in range(lo, hi):
                emit_csum_mm(b, i)
            if hi < ST:
                return
            cs_lo, cs_hi = st["cs_lo"], st["cs_hi"]
            csum = front.tile([64, D], f16, tag="csum")
            nc.vector.tensor_copy(csum[:, 0:512], cs_lo[:])
            nc.vector.tensor_copy(csum[:, 512:1024], cs_hi[:])

            # CavgT packed [128, (d n)] via 8 PE transposes
            csumt = front.tile([128, DT * NSEG], f16, tag="csumt")
            pt = ps_t.tile([128, 1024], f16, tag="tp")
            pt2 = ps_t.tile([128, 1024], f16, tag="tp")
            for d in range(DT):
                po = pt if d < 4 else pt2
                nc.tensor.transpose(po[:, 64 * (d % 4):64 * (d % 4 + 1)],
                                    csum[0:64, 128 * d:128 * (d + 1)],
                                    identh[0:64, 0:64])
            nc.vector.tensor_copy(csumt[:, 0:256], pt[:, 0:256])
            nc.vector.tensor_copy(csumt[:, 256:512], pt2[:, 0:256])
            st["csumt"] = csumt
            csumt8 = front.tile([128, DT * NSEG], f8, tag="csumt8")
            nc.vector.tensor_scalar_mul(csumt8[:, 0:256], pt[:, 0:256], 16.0)
            nc.vector.tensor_scalar_mul(csumt8[:, 256:512], pt2[:, 0:256], 16.0)
            st["csumt8"] = csumt8

            urt = front.tile([65, Q], f16, tag="urt")
            nc.vector.memset(urt[64:65, :], 1.0)
            st["urt"] = urt

        def emit_segavg(b):
            # segavgT[n, q] = Cavg @ O^T as 8 wide matmuls, then transpose to
            # [q, n] and run all four q-tiles' softmaxes up front (decoupled
            # from the O@W2 stream).
            st = state[b]
            csumt, ott, lnc = st["csumt"], st["ott"], st["lnc"]
            sgt = ps64.tile([64, Q], f32, tag="a64")
            for d in range(DT):
                nc.tensor.matmul(sgt[:], csumt[:, NSEG * d:NSEG * (d + 1)],
                                 ott[:, Q * d:Q * (d + 1)],
                                 start=(d == 0), stop=(d == DT - 1))
            sgt_sb = front.tile([64, Q], f16, tag="sgt")
            nc.vector.tensor_copy(sgt_sb[:], sgt[:])
            sgtT = ps_t.tile([128, 1024], f16, tag="tp")
            for j in range(QT):
                nc.tensor.transpose(sgtT[:, 64 * j:64 * (j + 1)],
                                    sgt_sb[0:64, 128 * j:128 * (j + 1)],
                                    identh[0:64, 0:64])
            for j in range(QT):
                sg2 = sm.tile([128, NSEG], f32, tag="sg2")
                nc.vector.tensor_add(sg2[:], sgtT[:, 64 * j:64 * (j + 1)], lnc[:])
                mx = sm.tile([128, 1], f32, tag="mx")
                nc.vector.reduce_max(mx[:], sg2[:], axis=Xax)
                negmx = sm.tile([128, 1], f32, tag="negmx")
                nc.vector.tensor_scalar_mul(negmx[:], mx[:], -1.0)
                u = sm.tile([128, NSEG], f16, tag="u")
                dsum = sm.tile([128, 1], f32, tag="dsum")
                nc.scalar.activation(u[:], sg2[:], Exp, bias=negmx[:],
                                     accum_out=dsum[:])
                rd = sm.tile([128, 1], f32, tag="rd")
                nc.vector.reciprocal(rd[:], dsum[:])
                urn = sm.tile([128, NSEG], f16, tag="urn", bufs=5)
                nc.vector.tensor_scalar_mul(urn[:], u[:], rd[:])
                nc.scalar.dma_start(urn_o.ap()[b, 128 * j:128 * (j + 1), :], urn[:])
                st[f"urn{j}"] = urn

        def emit_urt(b):
            # urT for the K=65 apply matmuls, all four q-tiles at once;
            # emitted a couple of qlo blocks late so the PE never waits on
            # the softmax chain.
            st = state[b]
            urt = st["urt"]
            pu = ps_t.tile([128, 1024], f16, tag="tp")
            for j in range(QT):
                nc.tensor.transpose(pu[0:64, 128 * j:128 * (j + 1)],
                                    st[f"urn{j}"][:], identh[:])
            nc.vector.tensor_copy(urt[0:64, :], pu[0:64, 0:512])

        def emit_p(b):
            st = state[b]
            csumt = st["csumt8"]
            # P_aug[0:64] = Cavg @ W1 ; row 64 = bias
            p_lo = ps64.tile([64, 512], f32, tag="a64")
            p_hi = ps64.tile([64, 512], f32, tag="a64")
            for d in range(DT):
                ct_d = csumt[:, NSEG * d:NSEG * (d + 1)]
                nc.tensor.matmul(p_lo[:], ct_d, w1_all[:, D * d:D * d + 512],
                                 start=(d == 0), stop=(d == DT - 1))
                nc.tensor.matmul(p_hi[:], ct_d, w1_all[:, D * d + 512:D * (d + 1)],
                                 start=(d == 0), stop=(d == DT - 1))
            paug = front.tile([65, D], f16, tag="paug")
            nc.vector.tensor_scalar_mul(paug[0:64, 0:512], p_lo[:], 1.0 / 4096.0)
            nc.vector.tensor_scalar_mul(paug[0:64, 512:1024], p_hi[:], 1.0 / 4096.0)
            nc.vector.tensor_copy(paug[64:65, :], bias_sb[:])
            st["paug"] = paug

        def emit_qlo(b, j):
            st = state[b]
            ott = st["ott"]
            o_lo = ps_lo.tile([128, 512], f32, tag="po_lo")
            for d in range(DT):
                otd = ott[:, Q * d + 128 * j:Q * d + 128 * (j + 1)]
                nc.tensor.matmul(o_lo[:], otd, w2_all[:, D * d:D * d + 512],
                                 start=(d == 0), stop=False)
            st[f"q{j}"] = o_lo

        def emit_qhi(b, j):
            st = state[b]
            ott = st["ott"]
            o_hi = ps_hi.tile([128, 512], f32, tag="po_hi")
            for d in range(DT):
                otd = ott[:, Q * d + 128 * j:Q * d + 128 * (j + 1)]
                nc.tensor.matmul(o_hi[:], otd, w2_all[:, D * d + 512:D * (d + 1)],
                                 start=(d == 0), stop=False)
            st[f"qh{j}"] = o_hi

        def emit_aplo(b, j):
            st = state[b]
            o_lo = st[f"q{j}"]
            urt, paug = st["urt"], st["paug"]
            ua = urt[:, 128 * j:128 * (j + 1)]
            nc.tensor.matmul(o_lo[:], ua, paug[:, 0:512], start=False, stop=True)
            ost = stage.tile([128, 512], f16, tag="ostl")
            nc.scalar.activation(ost[:], o_lo[:], Tanh)
            nc.scalar.dma_start(out_o.ap()[b, 128 * j:128 * (j + 1), 0:512], ost[:])

        def emit_aphi(b, j):
            st = state[b]
            o_hi = st[f"qh{j}"]
            urt, paug = st["urt"], st["paug"]
            ua = urt[:, 128 * j:128 * (j + 1)]
            nc.tensor.matmul(o_hi[:], ua, paug[:, 512:1024], start=False, stop=True)
            ost = stage.tile([128, 512], f16, tag="osth")
            nc.scalar.activation(ost[:], o_hi[:], Tanh)
            nc.scalar.dma_start(out_o.ap()[b, 128 * j:128 * (j + 1), 512:1024], ost[:])

        # ---- emission ----
        # sync ring: batch-0 ctx (256KB chunks for steady HAM-safe cadence) +
        # O^T, then w2-hi, then the batch-1 bulk.  scalar ring (slow at
        # bulk): early batch-1 ctx head.
        cv0, cs0 = emit_loads(0, nc.sync)
        for t in range(ST):
            nc.sync.dma_start(cv0[:, t:t + 1, :], cs0[:, t:t + 1, :])
        emit_load_ott(0, nc.sync)
        nc.sync.dma_start(w2v[:, 0:4, 512:1024], w2s[:, 0:4, 512:1024])
        nc.sync.dma_start(w2v[:, 4:8, 512:1024], w2s[:, 4:8, 512:1024])
        cv1, cs1 = emit_loads(1, nc.scalar)
        for t in range(6):
            nc.scalar.dma_start(cv1[:, t:t + 1, :], cs1[:, t:t + 1, :])
        for t in range(6, 8):
            nc.sync.dma_start(cv1[:, t:t + 1, :], cs1[:, t:t + 1, :])
        emit_load_ott(1, nc.sync)

        emit_warmup()
        emit_csum(0)
        emit_segavg(0)
        emit_rewarm(16)
        emit_qlo(0, 0)
        emit_qlo(0, 1)
        emit_urt(0)
        emit_p(0)
        emit_aplo(0, 0)
        emit_qlo(0, 2)
        emit_aplo(0, 1)
        emit_qlo(0, 3)
        emit_aplo(0, 2)
        emit_aplo(0, 3)
        emit_qhi(0, 0)
        emit_csum(1, 0, 2)
        emit_aphi(0, 0)
        emit_qhi(0, 1)
        emit_csum(1, 2, 4)
        emit_aphi(0, 1)
        emit_qhi(0, 2)
        emit_csum(1, 4, 6)
        emit_aphi(0, 2)
        emit_qhi(0, 3)
        emit_csum(1, 6, 8)
        emit_aphi(0, 3)
        emit_segavg(1)
        emit_p(1)
        emit_qlo(1, 0)
        emit_qlo(1, 1)
        emit_urt(1)
        emit_aplo(1, 0)
        emit_qlo(1, 2)
        emit_aplo(1, 1)
        emit_qlo(1, 3)
        emit_aplo(1, 2)
        emit_aplo(1, 3)
        emit_qhi(1, 0)
        emit_aphi(1, 0)
        emit_qhi(1, 1)
        emit_aphi(1, 1)
        emit_qhi(1, 2)
        emit_aphi(1, 2)
        emit_qhi(1, 3)
        emit_aphi(1, 3)

    nc.compile()
    return nc


def _host_prep(output, context, W_weight, W_bias, segment_ids):
    """Shard over batch; fp16 conversion + index/layout prep (no reductions)."""
    import concourse.mybir as mybir
    np_f8 = mybir.dt.np(mybir.dt.float8e4)
    wt = W_weight.T.astype(np.float16)                       # [2D, D]
    w1 = np.ascontiguousarray((wt[:D].astype(np.float32) * 256.0).astype(np_f8))
    w2 = np.ascontiguousarray(wt[D:])
    biasr = np.ascontiguousarray(W_bias.astype(np.float16)[None, :])
    identh = np.eye(128, dtype=np.float16)

    in_maps, aligns = [], []
    for c in range(NCORES):
        lo = c * BPC
        ohis, lncs, invcs = [], [], []
        for b in range(BPC):
            ids = segment_ids[lo + b].astype(np.int64)       # [S]
            oh = (ids[:, None] == np.arange(NSEG)[None, :]).astype(np.float32)
            cnt = oh.sum(axis=0)                             # [NSEG]
            invc = 1.0 / np.maximum(cnt, 1.0)
            ohi = (oh * invc[None, :]).astype(np.float16)    # [S, NSEG]
            ohis.append(np.ascontiguousarray(
                ohi.reshape(ST, 128, NSEG).transpose(1, 0, 2).reshape(128, ST * NSEG)))
            lnrow = np.where(cnt > 0, np.log(np.maximum(cnt, 1.0)), -1e30)
            lncs.append(np.ascontiguousarray(np.broadcast_to(
                lnrow.astype(np.float32)[None, :], (128, NSEG))))
            invcs.append(invc)
        in_maps.append({
            "ctx_in": np.ascontiguousarray(context[lo:lo + BPC].astype(np.float16)),
            "ott_in": np.ascontiguousarray(
                output[lo:lo + BPC].astype(np.float16).transpose(0, 2, 1)),
            "w1_in": w1, "w2_in": w2, "bias_in": biasr, "identh_in": identh,
            "ohi_in": np.stack(ohis), "lnc_in": np.stack(lncs),
        })
        aligns.append(invcs)
    return in_maps, aligns


def _run(inputs, trace=False, tmpdir=None):
    from concourse.bass_utils import run_bass_kernel_spmd
    if "nc" not in _CACHE:
        _CACHE["nc"] = _build_nc()
    nc = _CACHE["nc"]
    in_maps, invcs = _host_prep(**inputs)
    kw = {}
    if trace:
        kw = {"trace": True, "tmpdir": tmpdir}
    res = run_bass_kernel_spmd(nc, in_maps, core_ids=list(range(NCORES)), **kw)
    out = np.concatenate(
        [res.results[c]["out_o"].astype(np.float32) for c in range(NCORES)], axis=0)
    # align[q, s] = urn[q, seg(s)] * invc[seg(s)]  — host-side gather/unshard
    seg = inputs["segment_ids"]
    align = np.empty((B, Q, S), dtype=np.float32)
    for c in range(NCORES):
        for b in range(BPC):
            gb = c * BPC + b
            urn = res.results[c]["urn_o"][b].astype(np.float32)   # [Q, NSEG]
            scaled = urn * invcs[c][b][None, :].astype(np.float32)
            align[gb] = scaled[:, seg[gb].astype(np.int64)]
    return (out, align), res


def kernel(output, context, W_weight, W_bias, segment_ids):
    # Force host numpy up front: if the caller hands us jax arrays, numpy
    # ops would otherwise dispatch to the accelerator backend.
    (out, align), _ = _run(dict(
        output=np.asarray(output, dtype=np.float32),
        context=np.asarray(context, dtype=np.float32),
        W_weight=np.asarray(W_weight, dtype=np.float32),
        W_bias=np.asarray(W_bias, dtype=np.float32),
        segment_ids=np.asarray(segment_ids, dtype=np.int32)))
    return out, align


# revision 72
# speedup vs baseline: 1.0564x; 1.0022x over previous
"""Trainium2 Bass kernel for nn_Attn_30820685316537 (segment_reduce attention).

Reference computation (per batch b):
    score = output @ context^T                     [Q, S]
    avg   = per-segment mean of score over S, broadcast back
    align = softmax(avg, axis=S)                   [Q, S]
    ac    = align @ context                        [Q, D]
    out   = tanh(concat(ac, output) @ W^T + bias)  [Q, D]
    returns (out, align)

Everything factors through rank-64 segment space (avg is constant within each
contiguous segment).  With Cavg[n, d] = (1/cnt_n) * sum_{s in seg n} C[s, d]:
    segavg[q, n] = O[q, :] . Cavg[n, :]
    u[q, n]      = exp(segavg + ln cnt_n - max)        (cnt-weighted softmax)
    urn[q, n]    = u / sum_n u                         (per-segment align mass)
    align[q, s]  = urn[q, seg(s)] / cnt_{seg(s)}       (host-side gather)
    ac[q, :]     = urn @ Cavg
    out          = tanh(urn @ (Cavg @ W1) + O @ W2 + bias)
where W^T = [W1; W2].  P = Cavg @ W1 is a [64, D] matrix, so the output GEMM
is O @ W2 (K=1024) plus a K=65 rank-64 correction (bias folded in as P's 65th
row) instead of the K=2048 concat GEMM.

Device schedule per batch:
  front: Cavg  = ohi^T @ ctx (one-hot matmul, invc pre-folded on host),
         CavgT = 8 PE transposes, segavgT = CavgT^T @ O^T as 8 wide (N=512)
         matmuls, 4 transposes back to [q, n], then all four q-tiles'
         softmaxes run up front on DVE/ACT (decoupled from the GEMM stream),
         P = CavgT^T @ W1 in fp8 (x4096 scaling keeps fp8 normals).
  qloop (per 128-row q-tile): O@W2 lo columns, then hi columns (the W2
         column-halves stream from HBM in that order); urn is transposed
         (batched, one PSUM tile) and applied (K=65) into the same PSUM
         accumulators; tanh + DMA out per half.
Two batches per core are software-pipelined; GEMMs fp16 (fp32 PSUM
accumulation), P-path fp8.  align is reconstructed on the host by a pure
gather of the device-computed urn masses.  DMA rings are byte-balanced and
ordered by need-time (ctx0/W early, batch-1 inputs behind batch-0 on the
same queues); warm-up/keep-alive matmuls hold the PE's HAM clock-gate at
full rate through the input-paced ramp.

Sharding: data-parallel over batch B=16 across 8 NeuronCores; W replicated.
"""
import numpy as np
from contextlib import ExitStack

B, Q, S, D = 16, 512, 1024, 1024
NSEG = 64
NCORES = 8
BPC = B // NCORES          # batches per core
QT = Q // 128              # 4 q-tiles
ST = S // 128              # 8 s-chunks
DT = D // 128              # 8 d-chunks

_CACHE = {}


def _build_nc():
    import concourse.bacc as bacc
    import concourse.tile as tile
    import concourse.mybir as mybir

    f32 = mybir.dt.float32
    f16 = mybir.dt.float16
    f8 = mybir.dt.float8e4

    nc = bacc.Bacc("TRN2", target_bir_lowering=False, debug=False,
                   enable_asserts=False, num_devices=NCORES)

    ctx_in = nc.dram_tensor("ctx_in", [BPC, S, D], f16, kind="ExternalInput")
    ott_in = nc.dram_tensor("ott_in", [BPC, D, Q], f16, kind="ExternalInput")   # O^T
    w1_in = nc.dram_tensor("w1_in", [D, D], f8, kind="ExternalInput")           # WT[:D]
    w2_in = nc.dram_tensor("w2_in", [D, D], f16, kind="ExternalInput")          # WT[D:]
    ohi_in = nc.dram_tensor("ohi_in", [BPC, 128, ST * NSEG], f16, kind="ExternalInput")
    lnc_in = nc.dram_tensor("lnc_in", [BPC, 128, NSEG], f32, kind="ExternalInput")
    bias_in = nc.dram_tensor("bias_in", [1, D], f16, kind="ExternalInput")
    identh_in = nc.dram_tensor("identh_in", [128, 128], f16, kind="ExternalInput")

    out_o = nc.dram_tensor("out_o", [BPC, Q, D], f16, kind="ExternalOutput")
    urn_o = nc.dram_tensor("urn_o", [BPC, Q, NSEG], f16, kind="ExternalOutput")

    Exp = mybir.ActivationFunctionType.Exp
    Tanh = mybir.ActivationFunctionType.Tanh

    with tile.TileContext(nc) as tc, ExitStack() as ectx:
        consts = ectx.enter_context(tc.tile_pool(name="consts", bufs=1))
        inp = ectx.enter_context(tc.tile_pool(name="inp", bufs=2))
        front = ectx.enter_context(tc.tile_pool(name="front", bufs=2))
        sm = ectx.enter_context(tc.tile_pool(name="sm", bufs=3))
        stage = ectx.enter_context(tc.tile_pool(name="stage", bufs=2))

        # PSUM: exactly 8 banks (2 + 1 + 2 + 2 + 1 junk).
        ps64 = ectx.enter_context(tc.tile_pool(name="ps64", bufs=2, space="PSUM"))
        ps_t = ectx.enter_context(tc.tile_pool(name="ps_t", bufs=1, space="PSUM"))
        ps_lo = ectx.enter_context(tc.tile_pool(name="ps_lo", bufs=2, space="PSUM"))
        ps_hi = ectx.enter_context(tc.tile_pool(name="ps_hi", bufs=2, space="PSUM"))
        ps_j = ectx.enter_context(tc.tile_pool(name="ps_j", bufs=1, space="PSUM"))

        # ---- const loads (gpsimd ring: identh first, then W1, then W2-lo) ----
        identh = consts.tile([128, 128], f16, tag="identh")
        nc.gpsimd.dma_start(identh[:], identh_in.ap())
        bias_sb = consts.tile([1, D], f16, tag="bias")
        nc.gpsimd.dma_start(bias_sb[:], bias_in.ap())

        w1_all = consts.tile([128, DT * D], f8, tag="w1")      # [p, (d f)] fp8
        w1v = w1_all[:].rearrange("p (c f) -> p c f", f=D)
        w1s = w1_in.ap().rearrange("(c p) f -> p c f", p=128)
        for t in range(2):
            nc.gpsimd.dma_start(w1v[:, 4 * t:4 * (t + 1), :], w1s[:, 4 * t:4 * (t + 1), :])

        w2_all = consts.tile([128, DT * D], f16, tag="w2")
        w2v = w2_all[:].rearrange("p (c f) -> p c f", f=D)
        w2s = w2_in.ap().rearrange("(c p) f -> p c f", p=128)
        # lo columns of every chunk first (feeds the qlo pass ~6µs before
        # the qhi pass needs the hi columns; w2-hi rides the sync ring).
        nc.gpsimd.dma_start(w2v[:, 0:4, 0:512], w2s[:, 0:4, 0:512])
        nc.gpsimd.dma_start(w2v[:, 4:8, 0:512], w2s[:, 4:8, 0:512])

        Xax = mybir.AxisListType.X

        state = [dict() for _ in range(BPC)]

        def emit_loads(b, eng):
            st = state[b]
            ohi = inp.tile([128, ST * NSEG], f16, tag="ohi")
            eng.dma_start(ohi[:], ohi_in.ap()[b])
            lnc = inp.tile([128, NSEG], f32, tag="lnc")
            eng.dma_start(lnc[:], lnc_in.ap()[b])
            ctx_all = inp.tile([128, ST * D], f16, tag="ctx")   # [p, (i d)]
            cv = ctx_all[:].rearrange("p (c d) -> p c d", d=D)
            cs = ctx_in.ap()[b].rearrange("(c p) d -> p c d", p=128)
            st["ohi"], st["lnc"], st["ctx"] = ohi, lnc, ctx_all
            return cv, cs

        def emit_load_ott(b, eng):
            st = state[b]
            ott_all = inp.tile([128, DT * Q], f16, tag="ott")   # [p, (d q)]
            ov = ott_all[:].rearrange("p (c q) -> p c q", q=Q)
            os_ = ott_in.ap()[b].rearrange("(c p) q -> p c q", p=128)
            eng.dma_start(ov[:, 0:4, :], os_[:, 0:4, :])
            eng.dma_start(ov[:, 4:8, :], os_[:, 4:8, :])
            st["ott"] = ott_all

        junk = [None]

        def emit_warmup():
            # Dense PE work during the initial DMA wait so HAM un-throttles
            # before the real stream starts (identh arrives in the first µs).
            junk[0] = ps_j.tile([128, 512], f32, tag="junk", name="junkpw")
            pw = junk[0]
            for r in range(32):
                nc.tensor.matmul(pw[:, 0:128], identh[:], identh[:],
                                 start=(r == 0), stop=(r == 31))

        def emit_rewarm(n):
            # Keep-alive matmuls emitted just before a known input-wait point:
            # they execute during the DMA wait and flip HAM back to full clock
            # before the dense GEMM stream starts.
            pw = junk[0]
            for r in range(n):
                nc.tensor.matmul(pw[:, 0:128], identh[:], identh[:],
                                 start=True, stop=True)

        def emit_csum_mm(b, i):
            st = state[b]
            ohi, ctx_all = st["ohi"], st["ctx"]
            if i == 0:
                st["cs_lo"] = ps64.tile([64, 512], f32, tag="a64", name="cs_lo")
                st["cs_hi"] = ps64.tile([64, 512], f32, tag="a64", name="cs_hi")
            oh_i = ohi[:, NSEG * i:NSEG * (i + 1)]
            nc.tensor.matmul(st["cs_lo"][:], oh_i, ctx_all[:, D * i:D * i + 512],
                             start=(i == 0), stop=(i == ST - 1))
            nc.tensor.matmul(st["cs_hi"][:], oh_i, ctx_all[:, D * i + 512:D * (i + 1)],
                             start=(i == 0), stop=(i == ST - 1))

        def emit_csum(b, lo=0, hi=ST):
            st = state[b]
            for i in range(lo, hi):
                emit_csum_mm(b, i)
            if hi < ST:
                return
            cs_lo, cs_hi = st["cs_lo"], st["cs_hi"]
            csum = front.tile([64, D], f16, tag="csum")
            nc.vector.tensor_copy(csum[:, 0:512], cs_lo[:])
            nc.vector.tensor_copy(csum[:, 512:1024], cs_hi[:])

            # CavgT packed [128, (d n)] via 8 PE transposes
            csumt = front.tile([128, DT * NSEG], f16, tag="csumt")
            pt = ps_t.tile([128, 1024], f16, tag="tp")
            pt2 = ps_t.tile([128, 1024], f16, tag="tp")
            for d in range(DT):
                po = pt if d < 4 else pt2
                nc.tensor.transpose(po[:, 64 * (d % 4):64 * (d % 4 + 1)],
                                    csum[0:64, 128 * d:128 * (d + 1)],
                                    identh[0:64, 0:64])
            nc.vector.tensor_copy(csumt[:, 0:256], pt[:, 0:256])
            nc.vector.tensor_copy(csumt[:, 256:512], pt2[:, 0:256])
            st["csumt"] = csumt
            csumt8 = front.tile([128, DT * NSEG], f8, tag="csumt8")
            nc.vector.tensor_scalar_mul(csumt8[:, 0:256], pt[:, 0:256], 16.0)
            nc.vector.tensor_scalar_mul(csumt8[:, 256:512], pt2[:, 0:256], 16.0)
            st["csumt8"] = csumt8

            urt = front.tile([65, Q], f16, tag="urt")
            nc.vector.memset(urt[64:65, :], 1.0)
            st["urt"] = urt

        def emit_segavg(b):
            # segavgT[n, q] = Cavg @ O^T as 8 wide matmuls, then transpose to
            # [q, n] and run all four q-tiles' softmaxes up front (decoupled
            # from the O@W2 stream).
            st = state[b]
            csumt, ott, lnc = st["csumt"], st["ott"], st["lnc"]
            sgt = ps64.tile([64, Q], f32, tag="a64")
            for d in range(DT):
                nc.tensor.matmul(sgt[:], csumt[:, NSEG * d:NSEG * (d + 1)],
                                 ott[:, Q * d:Q * (d + 1)],
                                 start=(d == 0), stop=(d == DT - 1))
            sgt_sb = front.tile([64, Q], f16, tag="sgt")
            nc.vector.tensor_copy(sgt_sb[:], sgt[:])
            sgtT = ps_t.tile([128, 1024], f16, tag="tp")
            for j in range(QT):
                nc.tensor.transpose(sgtT[:, 64 * j:64 * (j + 1)],
                                    sgt_sb[0:64, 128 * j:128 * (j + 1)],
                                    identh[0:64, 0:64])
            for j in range(QT):
                sg2 = sm.tile([128, NSEG], f32, tag="sg2")
                nc.vector.tensor_add(sg2[:], sgtT[:, 64 * j:64 * (j + 1)], lnc[:])
                mx = sm.tile([128, 1], f32, tag="mx")
                nc.vector.reduce_max(mx[:], sg2[:], axis=Xax)
                negmx = sm.tile([128, 1], f32, tag="negmx")
                nc.vector.tensor_scalar_mul(negmx[:], mx[:], -1.0)
                u = sm.tile([128, NSEG], f16, tag="u")
                dsum = sm.tile([128, 1], f32, tag="dsum")
                nc.scalar.activation(u[:], sg2[:], Exp, bias=negmx[:],
                                     accum_out=dsum[:])
                rd = sm.tile([128, 1], f32, tag="rd")
                nc.vector.reciprocal(rd[:], dsum[:])
                urn = sm.tile([128, NSEG], f16, tag="urn", bufs=5)
                nc.vector.tensor_scalar_mul(urn[:], u[:], rd[:])
                nc.scalar.dma_start(urn_o.ap()[b, 128 * j:128 * (j + 1), :], urn[:])
                st[f"urn{j}"] = urn

        def emit_urt(b):
            # urT for the K=65 apply matmuls, all four q-tiles at once;
            # emitted a couple of qlo blocks late so the PE never waits on
            # the softmax chain.
            st = state[b]
            urt = st["urt"]
            pu = ps_t.tile([128, 1024], f16, tag="tp")
            for j in range(QT):
                nc.tensor.transpose(pu[0:64, 128 * j:128 * (j + 1)],
                                    st[f"urn{j}"][:], identh[:])
            nc.vector.tensor_copy(urt[0:64, :], pu[0:64, 0:512])

        def emit_p(b):
            st = state[b]
            csumt = st["csumt8"]
            # P_aug[0:64] = Cavg @ W1 ; row 64 = bias
            p_lo = ps64.tile([64, 512], f32, tag="a64")
            p_hi = ps64.tile([64, 512], f32, tag="a64")
            for d in range(DT):
                ct_d = csumt[:, NSEG * d:NSEG * (d + 1)]
                nc.tensor.matmul(p_lo[:], ct_d, w1_all[:, D * d:D * d + 512],
                                 start=(d == 0), stop=(d == DT - 1))
                nc.tensor.matmul(p_hi[:], ct_d, w1_all[:, D * d + 512:D * (d + 1)],
                                 start=(d == 0), stop=(d == DT - 1))
            paug = front.tile([65, D], f16, tag="paug")
            nc.vector.tensor_scalar_mul(paug[0:64, 0:512], p_lo[:], 1.0 / 4096.0)
            nc.vector.tensor_scalar_mul(paug[0:64, 512:1024], p_hi[:], 1.0 / 4096.0)
            nc.vector.tensor_copy(paug[64:65, :], bias_sb[:])
            st["paug"] = paug

        def emit_qlo(b, j):
            st = state[b]
            ott = st["ott"]
            o_lo = ps_lo.tile([128, 512], f32, tag="po_lo")
            for d in range(DT):
                otd = ott[:, Q * d + 128 * j:Q * d + 128 * (j + 1)]
                nc.tensor.matmul(o_lo[:], otd, w2_all[:, D * d:D * d + 512],
                                 start=(d == 0), stop=False)
            st[f"q{j}"] = o_lo

        def emit_qhi(b, j):
            st = state[b]
            ott = st["ott"]
            o_hi = ps_hi.tile([128, 512], f32, tag="po_hi")
            for d in range(DT):
                otd = ott[:, Q * d + 128 * j:Q * d + 128 * (j + 1)]
                nc.tensor.matmul(o_hi[:], otd, w2_all[:, D * d + 512:D * (d + 1)],
                                 start=(d == 0), stop=False)
            st[f"qh{j}"] = o_hi

        def emit_aplo(b, j):
            st = state[b]
            o_lo = st[f"q{j}"]
            urt, paug = st["urt"], st["paug"]
            ua = urt[:, 128 * j:128 * (j + 1)]
            nc.tensor.matmul(o_lo[:], ua, paug[:, 0:512], start=False, stop=True)
            ost = stage.tile([128, 512], f16, tag="ostl")
            nc.scalar.activation(ost[:], o_lo[:], Tanh)
            nc.scalar.dma_start(out_o.ap()[b, 128 * j:128 * (j + 1), 0:512], ost[:])

        def emit_aphi(b, j):
            st = state[b]
            o_hi = st[f"qh{j}"]
            urt, paug = st["urt"], st["paug"]
            ua = urt[:, 128 * j:128 * (j + 1)]
            nc.tensor.matmul(o_hi[:], ua, paug[:, 512:1024], start=False, stop=True)
            ost = stage.tile([128, 512], f16, tag="osth")
            nc.scalar.activation(ost[:], o_hi[:], Tanh)
            nc.scalar.dma_start(out_o.ap()[b, 128 * j:128 * (j + 1), 512:1024], ost[:])

        # ---- emission ----
        # sync ring: batch-0 ctx (256KB chunks for steady HAM-safe cadence) +
        # O^T, then w2-hi, then the batch-1 bulk.  scalar ring (slow at
        # bulk): early batch-1 ctx head.
        cv0, cs0 = emit_loads(0, nc.sync)
        for t in range(ST):
            nc.sync.dma_start(cv0[:, t:t + 1, :], cs0[:, t:t + 1, :])
        emit_load_ott(0, nc.sync)
        nc.sync.dma_start(w2v[:, 0:4, 512:1024], w2s[:, 0:4, 512:1024])
        nc.sync.dma_start(w2v[:, 4:8, 512:1024], w2s[:, 4:8, 512:1024])
        cv1, cs1 = emit_loads(1, nc.scalar)
        for t in range(6):
            nc.scalar.dma_start(cv1[:, t:t + 1, :], cs1[:, t:t + 1, :])
        for t in range(6, 8):
            nc.sync.dma_start(cv1[:, t:t + 1, :], cs1[:, t:t + 1, :])
        emit_load_ott(1, nc.sync)

        emit_warmup()
        emit_csum(0)
        emit_segavg(0)
        emit_qlo(0, 0)
        emit_qlo(0, 1)
        emit_urt(0)
        emit_p(0)
        emit_aplo(0, 0)
        emit_qlo(0, 2)
        emit_aplo(0, 1)
        emit_qlo(0, 3)
        emit_aplo(0, 2)
        emit_aplo(0, 3)
        emit_qhi(0, 0)
        emit_csum(1, 0, 2)
        emit_aphi(0, 0)
        emit_qhi(0, 1)
        emit_csum(1, 2, 4)
        emit_aphi(0, 1)
        emit_qhi(0, 2)
        emit_csum(1, 4, 6)
        emit_aphi(0, 2)
        emit_qhi(0, 3)
        emit_csum(1, 6, 8)
        emit_aphi(0, 3)
        emit_segavg(1)
        emit_p(1)
        emit_qlo(1, 0)
        emit_qlo(1, 1)
        emit_urt(1)
        emit_aplo(1, 0)
        emit_qlo(1, 2)
        emit_aplo(1, 1)
        emit_qlo(1, 3)
        emit_aplo(1, 2)
        emit_aplo(1, 3)
        emit_qhi(1, 0)
        emit_aphi(1, 0)
        emit_qhi(1, 1)
        emit_aphi(1, 1)
        emit_qhi(1, 2)
        emit_aphi(1, 2)
        emit_qhi(1, 3)
        emit_aphi(1, 3)

    nc.compile()
    return nc


def _host_prep(output, context, W_weight, W_bias, segment_ids):
    """Shard over batch; fp16 conversion + index/layout prep (no reductions)."""
    import concourse.mybir as mybir
    np_f8 = mybir.dt.np(mybir.dt.float8e4)
    wt = W_weight.T.astype(np.float16)                       # [2D, D]
    w1 = np.ascontiguousarray((wt[:D].astype(np.float32) * 256.0).astype(np_f8))
    w2 = np.ascontiguousarray(wt[D:])
    biasr = np.ascontiguousarray(W_bias.astype(np.float16)[None, :])
    identh = np.eye(128, dtype=np.float16)

    in_maps, aligns = [], []
    for c in range(NCORES):
        lo = c * BPC
        ohis, lncs, invcs = [], [], []
        for b in range(BPC):
            ids = segment_ids[lo + b].astype(np.int64)       # [S]
            oh = (ids[:, None] == np.arange(NSEG)[None, :]).astype(np.float32)
            cnt = oh.sum(axis=0)                             # [NSEG]
            invc = 1.0 / np.maximum(cnt, 1.0)
            ohi = (oh * invc[None, :]).astype(np.float16)    # [S, NSEG]
            ohis.append(np.ascontiguousarray(
                ohi.reshape(ST, 128, NSEG).transpose(1, 0, 2).reshape(128, ST * NSEG)))
            lnrow = np.where(cnt > 0, np.log(np.maximum(cnt, 1.0)), -1e30)
            lncs.append(np.ascontiguousarray(np.broadcast_to(
                lnrow.astype(np.float32)[None, :], (128, NSEG))))
            invcs.append(invc)
        in_maps.append({
            "ctx_in": np.ascontiguousarray(context[lo:lo + BPC].astype(np.float16)),
            "ott_in": np.ascontiguousarray(
                output[lo:lo + BPC].astype(np.float16).transpose(0, 2, 1)),
            "w1_in": w1, "w2_in": w2, "bias_in": biasr, "identh_in": identh,
            "ohi_in": np.stack(ohis), "lnc_in": np.stack(lncs),
        })
        aligns.append(invcs)
    return in_maps, aligns


def _run(inputs, trace=False, tmpdir=None):
    from concourse.bass_utils import run_bass_kernel_spmd
    if "nc" not in _CACHE:
        _CACHE["nc"] = _build_nc()
    nc = _CACHE["nc"]
    in_maps, invcs = _host_prep(**inputs)
    kw = {}
    if trace:
        kw = {"trace": True, "tmpdir": tmpdir}
    res = run_bass_kernel_spmd(nc, in_maps, core_ids=list(range(NCORES)), **kw)
    out = np.concatenate(
        [res.results[c]["out_o"].astype(np.float32) for c in range(NCORES)], axis=0)
    # align[q, s] = urn[q, seg(s)] * invc[seg(s)]  — host-side gather/unshard
    seg = inputs["segment_ids"]
    align = np.empty((B, Q, S), dtype=np.float32)
    for c in range(NCORES):
        for b in range(BPC):
            gb = c * BPC + b
            urn = res.results[c]["urn_o"][b].astype(np.float32)   # [Q, NSEG]
            scaled = urn * invcs[c][b][None, :].astype(np.float32)
            align[gb] = scaled[:, seg[gb].astype(np.int64)]
    return (out, align), res


def kernel(output, context, W_weight, W_bias, segment_ids):
    # Force host numpy up front: if the caller hands us jax arrays, numpy
    # ops would otherwise dispatch to the accelerator backend.
    (out, align), _ = _run(dict(
        output=np.asarray(output, dtype=np.float32),
        context=np.asarray(context, dtype=np.float32),
        W_weight=np.asarray(W_weight, dtype=np.float32),
        W_bias=np.asarray(W_bias, dtype=np.float32),
        segment_ids=np.asarray(segment_ids, dtype=np.int32)))
    return out, align
